# revision 1
# baseline (speedup 1.0000x reference)
"""Trainium2 Bass kernel for nn_DiTBlock (HGRN-attention DiT block).

Sharding: 8 cores = 4 batches x 2 half-sequences (1024 tokens each).
All matmuls run as exact-integer bf16 matmuls (activations quantized to
int8-range integers stored in bf16; ternary weights quantized on host).
The time-recurrence h_t = f_t*h_{t-1} + i_t uses the DVE tensor_tensor_scan
instruction; the half-sequence boundary carry crosses cores via an
AllGather + one-hot mask matmul. adaln params are computed on-device,
sharded 8 ways over the 6144 outputs and AllGathered.
"""
import functools
import numpy as np
import ml_dtypes

import concourse.bass as bass
import concourse.bacc as bacc_mod
import concourse.mybir as mybir
import concourse.tile as tile
from concourse.masks import make_identity
from concourse.bass_utils import run_bass_kernel_spmd

BF16 = ml_dtypes.bfloat16
F32 = mybir.dt.float32
BF = mybir.dt.bfloat16
U32 = mybir.dt.uint32
AL = mybir.AluOpType
AF = mybir.ActivationFunctionType
AX = mybir.AxisListType

B, T, D = 4, 2048, 1024
TOK = 1024          # tokens per core
NH, HD = 16, 64
MLP = 4096
N_CORES = 8
C_MAGIC = float(1.5 * 2 ** 23)
MAGIC_U32 = 0x5F3759DF


def _quant_w(w):
    invws = float(np.clip(np.abs(w).mean(dtype=np.float64), 1e-5, None))
    m = np.clip(np.round(w.astype(np.float64) / invws), -1, 1).astype(np.float32)
    return np.ascontiguousarray(m.astype(BF16)), np.float32(invws)


def _rsqrt(nc, sb, x_ap, scale, bias, shape, tag):
    """out = rsqrt(x*scale + bias), Newton on DVE. Returns a new tile."""
    t = sb.tile(shape, F32, tag=tag + "_t", name=tag + "_t")
    nc.vector.tensor_scalar(out=t, in0=x_ap, scalar1=float(scale),
                            scalar2=float(bias), op0=AL.mult, op1=AL.add)
    y = sb.tile(shape, F32, tag=tag + "_y", name=tag + "_y")
    sh = sb.tile(shape, F32, tag=tag + "_s", name=tag + "_s")
    nc.vector.tensor_scalar(out=sh[:].bitcast(U32), in0=t[:].bitcast(U32),
                            scalar1=1, scalar2=None, op0=AL.logical_shift_right)
    mg = sb.tile(shape, F32, tag=tag + "_m", name=tag + "_m")
    nc.vector.memset(mg[:].bitcast(U32), MAGIC_U32)
    nc.vector.tensor_tensor(out=y[:].bitcast(U32), in0=mg[:].bitcast(U32),
                            in1=sh[:].bitcast(U32), op=AL.subtract)
    e = sb.tile(shape, F32, tag=tag + "_e", name=tag + "_e")
    for _ in range(3):
        nc.vector.tensor_tensor(out=e, in0=y, in1=y, op=AL.mult)
        nc.vector.tensor_tensor(out=e, in0=e, in1=t, op=AL.mult)
        nc.vector.tensor_scalar(out=e, in0=e, scalar1=-0.5, scalar2=1.5,
                                op0=AL.mult, op1=AL.add)
        nc.vector.tensor_tensor(out=y, in0=y, in1=e, op=AL.mult)
    return y


def _build(iw):
    """iw: dict of invws floats. Returns finalized Bacc program."""
    nc = bacc_mod.Bacc("TRN2", target_bir_lowering=False)

    x_sl = nc.declare_dram_parameter("x_sl", [TOK, D], F32, isOutput=False)
    c_cols = nc.declare_dram_parameter("c_cols", [128, 8, B], F32, isOutput=False)
    adw_sl = nc.declare_dram_parameter("adw_sl", [D, 768], F32, isOutput=False)
    adb_row = nc.declare_dram_parameter("adb_row", [1, 6 * D], F32, isOutput=False)
    mask8 = nc.declare_dram_parameter("mask8", [N_CORES, 1], F32, isOutput=False)
    bmask = nc.declare_dram_parameter("bmask", [B, 1], F32, isOutput=False)
    gnr = nc.declare_dram_parameter("gnr", [1, D], F32, isOutput=False)
    wiT = nc.declare_dram_parameter("wiT", [D, D], BF, isOutput=False)
    wfT = nc.declare_dram_parameter("wfT", [D, D], BF, isOutput=False)
    wgT = nc.declare_dram_parameter("wgT", [D, D], BF, isOutput=False)
    woT = nc.declare_dram_parameter("woT", [D, D], BF, isOutput=False)
    gwT = nc.declare_dram_parameter("gwT", [D, 2 * MLP], BF, isOutput=False)
    dwT = nc.declare_dram_parameter("dwT", [MLP, D], BF, isOutput=False)
    out_sl = nc.declare_dram_parameter("out_sl", [TOK, D], F32, isOutput=True)

    cc1_in = nc.dram_tensor("cc1_in", [B, 768], F32)
    cc1_out = nc.dram_tensor("cc1_out", [N_CORES * B, 768], F32, addr_space="Shared")
    cc2_in = nc.dram_tensor("cc2_in", [D], F32)
    cc2_out = nc.dram_tensor("cc2_out", [N_CORES, D], F32, addr_space="Shared")

    RG = [list(range(N_CORES))]

    with tile.TileContext(nc) as tc:
        import contextlib
        es = contextlib.ExitStack()
        with es:
            # pools
            cst = es.enter_context(tc.tile_pool(name="cst", bufs=1))
            ps = es.enter_context(tc.tile_pool(name="ps", bufs=1, space="PSUM"))
            dr = es.enter_context(tc.tile_pool(name="dr", bufs=1, space="DRAM"))
            pX2 = es.enter_context(tc.tile_pool(name="pX2", bufs=1))
            pOC = es.enter_context(tc.tile_pool(name="pOC", bufs=1))
            pA1 = es.enter_context(tc.tile_pool(name="pA1", bufs=1))
            p0ctx = tc.tile_pool(name="p0", bufs=2)
            wk = p0ctx.__enter__()

            def pmm(tag="mm"):
                return ps.tile([128, 512], F32, tag=tag, name=tag)

            # ---------------- phase 0: consts + adaln ----------------
            identb = cst.tile([128, 128], BF)
            make_identity(nc, identb)
            identf = cst.tile([128, 128], F32)
            make_identity(nc, identf)
            ones_row = cst.tile([1, 128], F32)
            nc.vector.memset(ones_row, 1.0)
            mask_sb = cst.tile([N_CORES, 1], F32)
            nc.sync.dma_start(out=mask_sb, in_=mask8[:, :])
            bmask_sb = cst.tile([B, 1], F32)
            nc.sync.dma_start(out=bmask_sb, in_=bmask[:, :])
            gnr_sb = cst.tile([1, D], F32)
            nc.sync.dma_start(out=gnr_sb, in_=gnr[:, :])
            adb_sb = wk.tile([1, 6 * D], F32, tag="adb", bufs=1)
            nc.sync.dma_start(out=adb_sb, in_=adb_row[:, :])

            c_sb = wk.tile([128, 8, B], F32, tag="csb")
            nc.sync.dma_start(out=c_sb, in_=c_cols[:, :, :])
            cs_sb = wk.tile([128, 8, B], F32, tag="cssb")
            nc.scalar.activation(out=cs_sb, in_=c_sb, func=AF.Silu)

            psA = ps.tile([B, 512], F32, tag="sm")
            psB = ps.tile([B, 256], F32, tag="sm2")
            for j in range(8):
                adw_j = wk.tile([128, 768], F32, tag="adw")
                nc.sync.dma_start(out=adw_j, in_=adw_sl[128 * j:128 * (j + 1), :])
                nc.tensor.matmul(psA, cs_sb[:, j, :], adw_j[:, 0:512],
                                 start=(j == 0), stop=(j == 7))
                nc.tensor.matmul(psB, cs_sb[:, j, :], adw_j[:, 512:768],
                                 start=(j == 0), stop=(j == 7))
            ad_sb = wk.tile([B, 768], F32, tag="adsb")
            nc.scalar.copy(out=ad_sb[:, 0:512], in_=psA)
            nc.scalar.copy(out=ad_sb[:, 512:768], in_=psB)
            nc.sync.dma_start(out=cc1_in[:, :], in_=ad_sb)
            nc.gpsimd.collective_compute(
                "AllGather", AL.bypass, ins=[cc1_in[:]], outs=[cc1_out[:]],
                replica_groups=RG)
            params_sb = wk.tile([1, 6 * D], F32, tag="params", bufs=1)
            for r in range(8):
                ag_r = wk.tile([B, 768], F32, tag="ag1")
                nc.sync.dma_start(out=ag_r, in_=cc1_out[4 * r:4 * (r + 1), :])
                pp1 = ps.tile([1, 512], F32, tag="sm")
                pp2 = ps.tile([1, 256], F32, tag="sm2")
                nc.tensor.matmul(pp1, bmask_sb, ag_r[:, 0:512], start=True, stop=True)
                nc.tensor.matmul(pp2, bmask_sb, ag_r[:, 512:768], start=True, stop=True)
                nc.scalar.copy(out=params_sb[:, 768 * r:768 * r + 512], in_=pp1)
                nc.scalar.copy(out=params_sb[:, 768 * r + 512:768 * (r + 1)], in_=pp2)
            nc.vector.tensor_tensor(out=params_sb, in0=params_sb, in1=adb_sb,
                                    op=AL.add)

            # broadcast the six modulation rows -> [128, D] tiles
            def bcast_row(pool, row_ap, bname, plus1=False, lhs=None):
                t = pool.tile([128, D], F32, tag=bname, name=bname)
                for ch in range(0, D, 512):
                    pb = pmm("bc")
                    nc.tensor.matmul(pb, ones_row if lhs is None else lhs,
                                     row_ap[:, ch:ch + 512], start=True, stop=True)
                    if plus1:
                        nc.scalar.activation(out=t[:, ch:ch + 512], in_=pb,
                                             func=AF.Identity, bias=1.0)
                    else:
                        nc.scalar.copy(out=t[:, ch:ch + 512], in_=pb)
                return t

            pr = params_sb.rearrange("one (six d) -> one six d", six=6)
            B_sh1 = bcast_row(cst, pr[:, 0, :], "Bsh1")
            B_sc1 = bcast_row(cst, pr[:, 1, :], "Bsc1", plus1=True)
            B_g1 = bcast_row(cst, pr[:, 2, :], "Bg1")
            B_sh2 = bcast_row(cst, pr[:, 3, :], "Bsh2")
            B_sc2 = bcast_row(cst, pr[:, 4, :], "Bsc2", plus1=True)
            B_g2 = bcast_row(cst, pr[:, 5, :], "Bg2")
            B_gn = bcast_row(cst, gnr_sb, "Bgn")

            # stat tiles (live long)
            q127A = cst.tile([128, 8], F32); dqA = cst.tile([128, 8], F32)
            dqAg = cst.tile([128, 8], F32)
            q127O = cst.tile([128, 8], F32); dqOo = cst.tile([128, 8], F32)
            q127C = cst.tile([128, 8], F32); dqCg = cst.tile([128, 8], F32)
            q127D = cst.tile([128, 8], F32); dqDo = cst.tile([128, 8], F32)
            Sb_i = pA1.tile([128, D], F32, bufs=1)
            Sb_f = pA1.tile([128, D], F32, bufs=1)
            xqT = pA1.tile([128, 8, D], BF, bufs=1)
            dqrow_d = dr.tile([D], F32, tag="dqrow")
            xnew_d = dr.tile([TOK, D], F32, tag="xnew")
            h2qT_d = dr.tile([32, 128, TOK], BF, tag="h2qT")
            h2_d = dr.tile([TOK, MLP], F32, tag="h2d")
            ca_d = dr.tile([TOK, TOK], F32, tag="cad")

            def quant_stats_sweep(src_get, n, amx, ssx, sb_pool, tagp):
                """absmax + sumsq per tile into amx/ssx cols."""
                for i in range(n):
                    s = src_get(i)
                    nc.vector.tensor_reduce(out=amx[:, i:i + 1], in_=s, axis=AX.X,
                                            op=AL.max, apply_absolute_value=True)
                    scr = sb_pool.tile([128, s.free_size()], F32, bufs=1,
                                       tag=tagp + "sq", name=tagp + "sq")
                    nc.scalar.activation(out=scr, in_=s, func=AF.Square,
                                         accum_out=ssx[:, i:i + 1])

            def quant_batch(amx, ssx, dk, q127, dqt, dq_scaled, iws_scaled,
                            sb_pool, tagp):
                """q127 = 127/max(amx,1e-5); dqt = amc*rsqrt(ssx/dk+1e-8)/127."""
                amc = sb_pool.tile([128, 8], F32, tag=tagp + "amc", name=tagp + "amc")
                nc.vector.tensor_scalar(out=amc, in0=amx, scalar1=1e-5,
                                        scalar2=None, op0=AL.max)
                rec = sb_pool.tile([128, 8], F32, tag=tagp + "rec", name=tagp + "rec")
                nc.vector.reciprocal(out=rec, in_=amc)
                nc.vector.tensor_scalar(out=q127, in0=rec, scalar1=127.0,
                                        scalar2=None, op0=AL.mult)
                rs = _rsqrt(nc, sb_pool, ssx, 1.0 / dk, 1e-8, [128, 8], tagp + "rs")
                nc.vector.tensor_tensor(out=dqt, in0=amc, in1=rs, op=AL.mult)
                nc.vector.tensor_scalar(out=dqt, in0=dqt, scalar1=1.0 / 127.0,
                                        scalar2=None, op0=AL.mult)
                if dq_scaled is not None:
                    nc.vector.tensor_scalar(out=dq_scaled, in0=dqt,
                                            scalar1=float(iws_scaled),
                                            scalar2=None, op0=AL.mult)

            def round_and_transpose(src, q_col, dst_bf, i, nblk, sb_pool, tagp,
                                    dram_out=None):
                """round src [128, 128*nblk] -> bf16, transpose blocks into
                dst_bf[:, j, 128i:...] (or spill to dram_out)."""
                w = 128 * nblk
                t2 = sb_pool.tile([128, w], F32, bufs=1, tag=tagp + "t2", name=tagp + "t2")
                nc.vector.tensor_scalar(out=t2, in0=src, scalar1=q_col,
                                        scalar2=C_MAGIC, op0=AL.mult, op1=AL.add)
                kq = sb_pool.tile([128, w], BF, bufs=1, tag=tagp + "kq", name=tagp + "kq")
                nc.vector.tensor_scalar(out=kq, in0=t2, scalar1=C_MAGIC,
                                        scalar2=None, op0=AL.subtract)
                for g4 in range(0, nblk, 4):
                    nb = min(4, nblk - g4)
                    tp = ps.tile([128, 512], BF, tag="tp", name="tp")
                    for jj in range(nb):
                        nc.tensor.transpose(tp[:, 128 * jj:128 * (jj + 1)],
                                            kq[:, 128 * (g4 + jj):128 * (g4 + jj + 1)],
                                            identb)
                    if dram_out is not None:
                        stg = sb_pool.tile([128, 512], BF, tag=tagp + "stg", name=tagp + "stg")
                        nc.scalar.copy(out=stg[:, 0:128 * nb], in_=tp[:, 0:128 * nb])
                        nc.sync.dma_start(
                            out=dram_out[g4:g4 + nb, :, 128 * i:128 * (i + 1)]
                            .rearrange("a p b -> p a b"),
                            in_=stg[:, 0:128 * nb].rearrange(
                                "p (b q) -> p b q", b=nb))
                    else:
                        for jj in range(nb):
                            eng = nc.scalar if (jj % 2 == 0) else nc.vector
                            if jj % 2 == 0:
                                nc.scalar.copy(
                                    out=dst_bf[:, g4 + jj, 128 * i:128 * (i + 1)],
                                    in_=tp[:, 128 * jj:128 * (jj + 1)])
                            else:
                                nc.vector.tensor_copy(
                                    out=dst_bf[:, g4 + jj, 128 * i:128 * (i + 1)],
                                    in_=tp[:, 128 * jj:128 * (jj + 1)])

            p0ctx.__exit__(None, None, None)

            # ---------------- phase A: LN + modulate + quant ----------------
            with tc.tile_pool(name="pa", bufs=2) as pa:
                muA = pa.tile([128, 8], F32, tag="muA")
                varA = pa.tile([128, 8], F32, tag="varA")
                amA = pa.tile([128, 8], F32, tag="amA")
                ssA = pa.tile([128, 8], F32, tag="ssA")
                moda = pa.tile([128, 8, D], F32, tag="moda", bufs=1)
                for i in range(8):
                    xi = pa.tile([128, D], F32, tag="xi")
                    nc.sync.dma_start(out=xi,
                                      in_=x_sl[128 * i:128 * (i + 1), :])
                    st = pa.tile([128, 2, 6], F32, tag="bst")
                    xr = xi.rearrange("p (s d) -> p s d", s=2)
                    for s2 in range(2):
                        nc.vector.bn_stats(out=st[:, s2, :], in_=xr[:, s2, :])
                    mv = pa.tile([128, 2], F32, tag="bmv")
                    nc.vector.bn_aggr(out=mv, in_=st)
                    nc.vector.tensor_copy(out=muA[:, i:i + 1], in_=mv[:, 0:1])
                    nc.vector.tensor_copy(out=varA[:, i:i + 1], in_=mv[:, 1:2])
                rstdLN = _rsqrt(nc, pa, varA, 1.0, 1e-6, [128, 8], "rLN")
                nmr = pa.tile([128, 8], F32, tag="nmr")
                nc.vector.tensor_tensor(out=nmr, in0=muA, in1=rstdLN, op=AL.mult)
                nc.vector.tensor_scalar(out=nmr, in0=nmr, scalar1=-1.0,
                                        scalar2=None, op0=AL.mult)
                for i in range(8):
                    xi = pa.tile([128, D], F32, tag="xi")
                    nc.sync.dma_start(out=xi,
                                      in_=x_sl[128 * i:128 * (i + 1), :])
                    u = pa.tile([128, D], F32, tag="u", bufs=1)
                    nc.scalar.activation(out=u, in_=xi, func=AF.Identity,
                                         scale=rstdLN[:, i:i + 1],
                                         bias=nmr[:, i:i + 1])
                    tt = pa.tile([128, D], F32, tag="tt", bufs=1)
                    nc.vector.tensor_tensor(out=tt, in0=u, in1=B_sc1, op=AL.mult)
                    nc.vector.tensor_tensor(out=moda[:, i, :], in0=tt, in1=B_sh1,
                                            op=AL.add)
                quant_stats_sweep(lambda i: moda[:, i, :], 8, amA, ssA, pa, "qa")
                quant_batch(amA, ssA, D, q127A, dqA, dqAg, iw["g"], pa, "qa")
                for i in range(8):
                    nc.sync.dma_start(
                        out=dqrow_d[128 * i:128 * (i + 1)].rearrange(
                            "(p one) -> p one", one=1),
                        in_=dqA[:, i:i + 1])
                dqrow_sb = pa.tile([1, D], F32, tag="dqrow")
                nc.sync.dma_start(out=dqrow_sb,
                                  in_=dqrow_d[:].rearrange("(one d) -> one d", one=1))
                oi = pa.tile([1, 128], F32, tag="oi")
                nc.vector.memset(oi, float(iw["i"]))
                of = pa.tile([1, 128], F32, tag="of")
                nc.vector.memset(of, float(iw["f"]))
                for ch in range(0, D, 512):
                    pb = pmm("bc")
                    nc.tensor.matmul(pb, oi, dqrow_sb[:, ch:ch + 512],
                                     start=True, stop=True)
                    nc.scalar.copy(out=Sb_i[:, ch:ch + 512], in_=pb)
                    pb2 = pmm("bc")
                    nc.tensor.matmul(pb2, of, dqrow_sb[:, ch:ch + 512],
                                     start=True, stop=True)
                    nc.scalar.copy(out=Sb_f[:, ch:ch + 512], in_=pb2)
                for i in range(8):
                    round_and_transpose(moda[:, i, :], q127A[:, i:i + 1], xqT,
                                        i, 8, pa, "ra")

            # ---------------- phase B: i/f matmuls + scan ----------------
            with tc.tile_pool(name="pb", bufs=2) as pb:
                ha = pb.tile([128, 8, TOK], F32, tag="ha", bufs=1)
                for m in range(8):
                    wf_m = pb.tile([128, 8, 128], BF, tag="wfm")
                    nc.sync.dma_start(
                        out=wf_m,
                        in_=wfT[:, 128 * m:128 * (m + 1)].rearrange(
                            "(a p) q -> p a q", p=128))
                    wi_m = pb.tile([128, 8, 128], BF, tag="wim")
                    nc.sync.dma_start(
                        out=wi_m,
                        in_=wiT[:, 128 * m:128 * (m + 1)].rearrange(
                            "(a p) q -> p a q", p=128))
                    ft = pb.tile([128, TOK], F32, tag="ftm", bufs=1)
                    it = pb.tile([128, TOK], F32, tag="itm", bufs=1)
                    for cki, ck in enumerate(range(0, TOK, 512)):
                        pf = pmm()
                        pi = pmm()
                        for j in range(8):
                            nc.tensor.matmul(pf, wf_m[:, j, :],
                                             xqT[:, j, ck:ck + 512],
                                             start=(j == 0), stop=(j == 7))
                        for j in range(8):
                            nc.tensor.matmul(pi, wi_m[:, j, :],
                                             xqT[:, j, ck:ck + 512],
                                             start=(j == 0), stop=(j == 7))
                        nc.vector.tensor_tensor(out=ft[:, ck:ck + 512], in0=pf,
                                                in1=Sb_f[:, ck:ck + 512], op=AL.mult)
                        nc.vector.tensor_tensor(out=it[:, ck:ck + 512], in0=pi,
                                                in1=Sb_i[:, ck:ck + 512], op=AL.mult)
                    sigf = pb.tile([128, TOK], F32, tag="sigf", bufs=1)
                    nc.scalar.activation(out=sigf, in_=ft, func=AF.Sigmoid)
                    sili = pb.tile([128, TOK], F32, tag="sili", bufs=1)
                    nc.scalar.activation(out=sili, in_=it, func=AF.Silu)
                    omf = pb.tile([128, TOK], F32, tag="ftm", bufs=1)
                    nc.vector.tensor_scalar(out=omf, in0=sigf, scalar1=-1.0,
                                            scalar2=1.0, op0=AL.mult, op1=AL.add)
                    ifin = pb.tile([128, TOK], F32, tag="itm", bufs=1)
                    nc.vector.tensor_tensor(out=ifin, in0=sili, in1=omf, op=AL.mult)
                    nc.vector.tensor_tensor_scan(ha[:, m, :], sigf, ifin, 0.0,
                                                 op0=AL.mult, op1=AL.add)
                    cam = pb.tile([128, TOK], F32, tag="cam", bufs=1)
                    nc.vector.tensor_tensor_scan(cam, sigf, sigf, 1.0,
                                                 op0=AL.mult, op1=AL.bypass)
                    nc.sync.dma_start(out=ca_d[128 * m:128 * (m + 1), :], in_=cam)
                    nc.sync.dma_start(
                        out=cc2_in[128 * m:128 * (m + 1)].rearrange(
                            "(p one) -> p one", one=1),
                        in_=ha[:, m, TOK - 1:TOK])
                nc.gpsimd.collective_compute(
                    "AllGather", AL.bypass, ins=[cc2_in[:]], outs=[cc2_out[:]],
                    replica_groups=RG)
                ag2 = pb.tile([N_CORES, D], F32, tag="ag2")
                nc.sync.dma_start(out=ag2, in_=cc2_out[:, :])

                # fixup + transpose h -> hT (still inside pb scope)
                hT = pA1.tile([128, 8, D], F32, tag="hT", bufs=1)
                pf = pb
                for m in range(8):
                    pc = ps.tile([128, 1], F32, tag="sm")
                    nc.tensor.matmul(pc, ag2[:, 128 * m:128 * (m + 1)], mask_sb,
                                     start=True, stop=True)
                    carry = pf.tile([128, 1], F32, tag="carry")
                    nc.scalar.copy(out=carry, in_=pc)
                    cam2 = pf.tile([128, TOK], F32, tag="cam2", bufs=1)
                    nc.sync.dma_start(out=cam2, in_=ca_d[128 * m:128 * (m + 1), :])
                    hfix = pf.tile([128, TOK], F32, tag="hfix", bufs=1)
                    nc.vector.scalar_tensor_tensor(out=hfix, in0=cam2,
                                                   scalar=carry, in1=ha[:, m, :],
                                                   op0=AL.mult, op1=AL.add)
                    for g4 in range(0, 8, 4):
                        tp = ps.tile([128, 512], F32, tag="tpf")
                        for jj in range(4):
                            t_i = g4 + jj
                            nc.tensor.transpose(tp[:, 128 * jj:128 * (jj + 1)],
                                                hfix[:, 128 * t_i:128 * (t_i + 1)],
                                                identf)
                        for jj in range(4):
                            t_i = g4 + jj
                            if jj % 2 == 0:
                                nc.scalar.copy(
                                    out=hT[:, t_i, 128 * m:128 * (m + 1)],
                                    in_=tp[:, 128 * jj:128 * (jj + 1)])
                            else:
                                nc.vector.tensor_copy(
                                    out=hT[:, t_i, 128 * m:128 * (m + 1)],
                                    in_=tp[:, 128 * jj:128 * (jj + 1)])

            # ---------------- o-stage ----------------
            oqT = pOC.tile([128, 8, D], BF, tag="oqT", bufs=1)
            with tc.tile_pool(name="po", bufs=2) as po:
                oa = po.tile([128, 8, D], F32, tag="oa", bufs=1)
                mshA = po.tile([128, 8, 16], F32, tag="msh", bufs=1)
                for t in range(8):
                    sq = po.tile([128, D], F32, tag="sq", bufs=1)
                    nc.vector.tensor_tensor(out=sq, in0=hT[:, t, :],
                                            in1=hT[:, t, :], op=AL.mult)
                    nc.vector.tensor_reduce(
                        out=mshA[:, t, :],
                        in_=sq.rearrange("p (h d) -> p h d", h=NH),
                        axis=AX.X, op=AL.add)
                rstdH = _rsqrt(nc, po, mshA[:, :, :].rearrange("p a b -> p (a b)"),
                               1.0 / HD, 1e-5, [128, 128], "rH")
                rH = rstdH.rearrange("p (a b) -> p a b", a=8)
                wg_sb = po.tile([128, 8, D], BF, tag="wgsb", bufs=1)
                for j in range(8):
                    nc.sync.dma_start(out=wg_sb[:, j, :],
                                      in_=wgT[128 * j:128 * (j + 1), :])
                amO = po.tile([128, 8], F32, tag="amO")
                ssO = po.tile([128, 8], F32, tag="ssO")
                for t in range(8):
                    gs = po.tile([128, D], F32, tag="gs", bufs=1)
                    for ck in range(0, D, 512):
                        pg = pmm()
                        for j in range(8):
                            nc.tensor.matmul(pg, xqT[:, j, 128 * t:128 * (t + 1)],
                                             wg_sb[:, j, ck:ck + 512],
                                             start=(j == 0), stop=(j == 7))
                        scr = po.tile([128, 512], F32, tag="gscr", bufs=2)
                        nc.scalar.activation(out=scr, in_=pg, func=AF.Silu,
                                             scale=dqAg[:, t:t + 1])
                        nc.vector.tensor_tensor(out=gs[:, ck:ck + 512], in0=scr,
                                                in1=B_gn[:, ck:ck + 512], op=AL.mult)
                    hn = po.tile([128, D], F32, tag="hn", bufs=1)
                    rb = bass.AP(tensor=rH.tensor, offset=rH[:, t, :].offset,
                                 ap=[rH.ap[0], [1, NH], [0, HD]])
                    nc.vector.tensor_tensor(
                        out=hn.rearrange("p (h d) -> p h d", h=NH),
                        in0=hT[:, t, :].rearrange("p (h d) -> p h d", h=NH),
                        in1=rb, op=AL.mult)
                    nc.vector.tensor_tensor(out=oa[:, t, :], in0=hn, in1=gs,
                                            op=AL.mult)
                quant_stats_sweep(lambda t: oa[:, t, :], 8, amO, ssO, po, "qo")
                quant_batch(amO, ssO, D, q127O, dqOo, dqOo, 1.0, po, "qo")
                nc.vector.tensor_scalar(out=dqOo, in0=dqOo, scalar1=float(iw["o"]),
                                        scalar2=None, op0=AL.mult)
                for t in range(8):
                    round_and_transpose(oa[:, t, :], q127O[:, t:t + 1], oqT,
                                        t, 8, po, "ro")

            # ---------------- phase C: wo matmul + residual + LN2 ----------
            x2qT = pX2.tile([128, 8, D], BF, tag="x2qT", bufs=1)
            with tc.tile_pool(name="pc", bufs=2) as pc:
                wo_sb = pc.tile([128, 8, D], BF, tag="wosb", bufs=1)
                for j in range(8):
                    nc.sync.dma_start(out=wo_sb[:, j, :],
                                      in_=woT[128 * j:128 * (j + 1), :])
                muC = pc.tile([128, 8], F32, tag="muC")
                varC = pc.tile([128, 8], F32, tag="varC")
                for t in range(8):
                    xr2 = pc.tile([128, D], F32, tag="xr2", bufs=1)
                    nc.sync.dma_start(out=xr2, in_=x_sl[128 * t:128 * (t + 1), :])
                    xn = pc.tile([128, D], F32, tag="xn", bufs=1)
                    for ck in range(0, D, 512):
                        pw = pmm()
                        for j in range(8):
                            nc.tensor.matmul(pw, oqT[:, j, 128 * t:128 * (t + 1)],
                                             wo_sb[:, j, ck:ck + 512],
                                             start=(j == 0), stop=(j == 7))
                        at = pc.tile([128, 512], F32, tag="at", bufs=1)
                        nc.vector.tensor_scalar(out=at, in0=pw,
                                                scalar1=dqOo[:, t:t + 1],
                                                scalar2=None, op0=AL.mult)
                        ug = pc.tile([128, 512], F32, tag="ug", bufs=1)
                        nc.vector.tensor_tensor(out=ug, in0=at,
                                                in1=B_g1[:, ck:ck + 512], op=AL.mult)
                        nc.vector.tensor_tensor(out=xn[:, ck:ck + 512], in0=ug,
                                                in1=xr2[:, ck:ck + 512], op=AL.add)
                    nc.sync.dma_start(out=xnew_d[128 * t:128 * (t + 1), :], in_=xn)
                    st = pc.tile([128, 2, 6], F32, tag="bst2")
                    xrr = xn.rearrange("p (s d) -> p s d", s=2)
                    for s2 in range(2):
                        nc.vector.bn_stats(out=st[:, s2, :], in_=xrr[:, s2, :])
                    mv = pc.tile([128, 2], F32, tag="bmv2")
                    nc.vector.bn_aggr(out=mv, in_=st)
                    nc.vector.tensor_copy(out=muC[:, t:t + 1], in_=mv[:, 0:1])
                    nc.vector.tensor_copy(out=varC[:, t:t + 1], in_=mv[:, 1:2])
                rstdC = _rsqrt(nc, pc, varC, 1.0, 1e-6, [128, 8], "rC")
                nmrC = pc.tile([128, 8], F32, tag="nmrC")
                nc.vector.tensor_tensor(out=nmrC, in0=muC, in1=rstdC, op=AL.mult)
                nc.vector.tensor_scalar(out=nmrC, in0=nmrC, scalar1=-1.0,
                                        scalar2=None, op0=AL.mult)
                amC = pc.tile([128, 8], F32, tag="amC")
                ssC = pc.tile([128, 8], F32, tag="ssC")
                mod2 = pc.tile([128, 8, D], F32, tag="mod2", bufs=1)
                for t in range(8):
                    xn2 = pc.tile([128, D], F32, tag="xn2", bufs=1)
                    nc.sync.dma_start(out=xn2, in_=xnew_d[128 * t:128 * (t + 1), :])
                    u = pc.tile([128, D], F32, tag="u2", bufs=1)
                    nc.scalar.activation(out=u, in_=xn2, func=AF.Identity,
                                         scale=rstdC[:, t:t + 1],
                                         bias=nmrC[:, t:t + 1])
                    tt2 = pc.tile([128, D], F32, tag="tt2", bufs=1)
                    nc.vector.tensor_tensor(out=tt2, in0=u, in1=B_sc2, op=AL.mult)
                    nc.vector.tensor_tensor(out=mod2[:, t, :], in0=tt2, in1=B_sh2,
                                            op=AL.add)
                quant_stats_sweep(lambda t: mod2[:, t, :], 8, amC, ssC, pc, "qc")
                quant_batch(amC, ssC, D, q127C, dqCg, dqCg, iw["gate"], pc, "qc")
                for t in range(8):
                    round_and_transpose(mod2[:, t, :], q127C[:, t:t + 1], x2qT,
                                        t, 8, pc, "rc")

            # ---------------- phase D: MLP ----------------
            with tc.tile_pool(name="pd", bufs=2) as pd:
                amDg = pd.tile([128, 8, 8], F32, tag="amDg")
                ssDg = pd.tile([128, 8, 8], F32, tag="ssDg")
                for g in range(8):
                    gw_g = pd.tile([128, 8, 1024], BF, tag="gwg", bufs=1)
                    for j in range(8):
                        nc.sync.dma_start(
                            out=gw_g[:, j, 0:512],
                            in_=gwT[128 * j:128 * (j + 1), 512 * g:512 * (g + 1)])
                        nc.sync.dma_start(
                            out=gw_g[:, j, 512:1024],
                            in_=gwT[128 * j:128 * (j + 1),
                                    MLP + 512 * g:MLP + 512 * (g + 1)])
                    for t in range(8):
                        pg = pmm()
                        py = pmm()
                        for j in range(8):
                            nc.tensor.matmul(pg, x2qT[:, j, 128 * t:128 * (t + 1)],
                                             gw_g[:, j, 0:512],
                                             start=(j == 0), stop=(j == 7))
                        for j in range(8):
                            nc.tensor.matmul(py, x2qT[:, j, 128 * t:128 * (t + 1)],
                                             gw_g[:, j, 512:1024],
                                             start=(j == 0), stop=(j == 7))
                        sil = pd.tile([128, 512], F32, tag="sil")
                        nc.scalar.activation(out=sil, in_=pg, func=AF.Silu,
                                             scale=dqCg[:, t:t + 1])
                        yv = pd.tile([128, 512], F32, tag="yv")
                        nc.vector.tensor_scalar(out=yv, in0=py,
                                                scalar1=dqCg[:, t:t + 1],
                                                scalar2=None, op0=AL.mult)
                        h2c = pd.tile([128, 512], F32, tag="h2c")
                        nc.vector.tensor_tensor(out=h2c, in0=sil, in1=yv,
                                                op=AL.mult)
                        nc.sync.dma_start(
                            out=h2_d[128 * t:128 * (t + 1),
                                     512 * g:512 * (g + 1)],
                            in_=h2c)
                        nc.vector.tensor_reduce(out=amDg[:, t, g:g + 1], in_=h2c,
                                                axis=AX.X, op=AL.max,
                                                apply_absolute_value=True)
                        scr = pd.tile([128, 512], F32, tag="sqd")
                        nc.scalar.activation(out=scr, in_=h2c, func=AF.Square,
                                             accum_out=ssDg[:, t, g:g + 1])
                amD = pd.tile([128, 8], F32, tag="amD")
                ssD = pd.tile([128, 8], F32, tag="ssD")
                nc.vector.tensor_reduce(out=amD, in_=amDg, axis=AX.X, op=AL.max)
                nc.vector.tensor_reduce(out=ssD, in_=ssDg, axis=AX.X, op=AL.add)
                quant_batch(amD, ssD, MLP, q127D, dqDo, dqDo, 1.0, pd, "qd")
                nc.vector.tensor_scalar(out=dqDo, in0=dqDo,
                                        scalar1=float(iw["down"]),
                                        scalar2=None, op0=AL.mult)
                for t in range(8):
                    h2r = pd.tile([128, MLP], F32, tag="h2r", bufs=1)
                    nc.sync.dma_start(out=h2r,
                                      in_=h2_d[128 * t:128 * (t + 1), :])
                    round_and_transpose(h2r, q127D[:, t:t + 1], None,
                                        t, 32, pd, "rd", dram_out=h2qT_d)

            with tc.tile_pool(name="pe2", bufs=2) as pe:
                dw_sb = pe.tile([128, 32, D], BF, tag="dwsb", bufs=1)
                for j2 in range(32):
                    nc.sync.dma_start(out=dw_sb[:, j2, :],
                                      in_=dwT[128 * j2:128 * (j2 + 1), :])
                for t in range(8):
                    h2t = pe.tile([128, 32, 128], BF, tag="h2t", bufs=1)
                    nc.sync.dma_start(out=h2t,
                                      in_=h2qT_d[:, :, 128 * t:128 * (t + 1)]
                                      .rearrange("a p b -> p a b"))
                    xn3 = pe.tile([128, D], F32, tag="xn3", bufs=1)
                    nc.sync.dma_start(out=xn3,
                                      in_=xnew_d[128 * t:128 * (t + 1), :])
                    outt = pe.tile([128, D], F32, tag="outt", bufs=1)
                    for ck in range(0, D, 512):
                        pdn = pmm()
                        for j2 in range(32):
                            nc.tensor.matmul(pdn, h2t[:, j2, :],
                                             dw_sb[:, j2, ck:ck + 512],
                                             start=(j2 == 0), stop=(j2 == 31))
                        u2 = pe.tile([128, 512], F32, tag="u2d", bufs=1)
                        nc.vector.tensor_scalar(out=u2, in0=pdn,
                                                scalar1=dqDo[:, t:t + 1],
                                                scalar2=None, op0=AL.mult)
                        v2 = pe.tile([128, 512], F32, tag="v2d", bufs=1)
                        nc.vector.tensor_tensor(out=v2, in0=u2,
                                                in1=B_g2[:, ck:ck + 512], op=AL.mult)
                        nc.vector.tensor_tensor(out=outt[:, ck:ck + 512], in0=v2,
                                                in1=xn3[:, ck:ck + 512], op=AL.add)
                    nc.sync.dma_start(out=out_sl[128 * t:128 * (t + 1), :], in_=outt)

    nc.finalize()
    return nc


@functools.lru_cache(maxsize=2)
def _build_cached(iw_items):
    return _build(dict(iw_items))


def kernel(x, c, adaln_w, adaln_b, wi, wf, wg, gnorm_w, wo, gate_w, down_w):
    x = np.ascontiguousarray(np.asarray(x, dtype=np.float32))
    c = np.ascontiguousarray(np.asarray(c, dtype=np.float32))
    adaln_w = np.asarray(adaln_w, dtype=np.float32)
    adaln_b = np.asarray(adaln_b, dtype=np.float32)
    gnorm_w = np.asarray(gnorm_w, dtype=np.float32)

    mi, iwi = _quant_w(np.asarray(wi, dtype=np.float32))
    mf, iwf = _quant_w(np.asarray(wf, dtype=np.float32))
    mg, iwg = _quant_w(np.asarray(wg, dtype=np.float32))
    mo, iwo = _quant_w(np.asarray(wo, dtype=np.float32))
    mgate, iwgate = _quant_w(np.asarray(gate_w, dtype=np.float32))
    mdown, iwdown = _quant_w(np.asarray(down_w, dtype=np.float32))

    iw = {"i": float(iwi), "f": float(iwf), "g": float(iwg), "o": float(iwo),
          "gate": float(iwgate), "down": float(iwdown)}
    nc = _build_cached(tuple(sorted(iw.items())))

    wiT_h = np.ascontiguousarray(mi.T)
    wfT_h = np.ascontiguousarray(mf.T)
    wgT_h = np.ascontiguousarray(mg.T)
    woT_h = np.ascontiguousarray(mo.T)
    gwT_h = np.ascontiguousarray(mgate.T)
    dwT_h = np.ascontiguousarray(mdown.T)
    adwT = np.ascontiguousarray(adaln_w.T)          # [D, 6D]
    adb_row_h = np.ascontiguousarray(adaln_b[None, :])
    gnr_h = np.ascontiguousarray(np.tile(gnorm_w, NH)[None, :])
    c_cols_h = np.ascontiguousarray(
        c.T.reshape(8, 128, B).transpose(1, 0, 2))   # [128, 8, B]

    in_maps = []
    for core in range(N_CORES):
        b, half = core // 2, core % 2
        mask = np.zeros((N_CORES, 1), np.float32)
        if half == 1:
            mask[core - 1, 0] = 1.0
        bm = np.zeros((B, 1), np.float32)
        bm[b, 0] = 1.0
        in_maps.append({
            "x_sl": np.ascontiguousarray(x[b, half * TOK:(half + 1) * TOK, :]),
            "c_cols": c_cols_h,
            "adw_sl": np.ascontiguousarray(adwT[:, 768 * core:768 * (core + 1)]),
            "adb_row": adb_row_h,
            "mask8": mask,
            "bmask": bm,
            "gnr": gnr_h,
            "wiT": wiT_h, "wfT": wfT_h, "wgT": wgT_h, "woT": woT_h,
            "gwT": gwT_h, "dwT": dwT_h,
        })

    res = run_bass_kernel_spmd(nc, in_maps, core_ids=list(range(N_CORES)))
    out = np.zeros((B, T, D), np.float32)
    for core in range(N_CORES):
        b, half = core // 2, core % 2
        out[b, half * TOK:(half + 1) * TOK, :] = res.results[core]["out_sl"]
    return out

